# revision 1
# baseline (speedup 1.0000x reference)
"""Trainium2 Bass kernel for nn_Mlp_cnn_shift (dense CNN MLP with 3x3 patch-shift
and a softmax-gated mix of two branches).

Strategy
--------
Data-parallel over the 16 (B,T) frames: each of the 8 NeuronCores processes 2
frames end-to-end.  All activations are kept channel-major ([C, tokens]) so the
channel contraction of every matmul has K on partitions, and `x` is
pre-transposed/cast on the host so no on-device transpose is needed.

Patch-shift handling:
 * forward shift (on xh, HID=1024): xh is stored in a zero-padded token layout
   (row pitch 57 = 56 cols + 1 zero pad col, 58-token zero guards per frame)
   and in 9 channel groups of 114 padded to 128 partitions each (host-permuted
   fc_w columns / fc1_w+fc2_w rows).  Every (dh,dw) roll then becomes a pure
   token offset in the fc1 matmul's rhs access pattern, with the zero padding
   reproducing the reference's zero-fill boundary exactly.
 * inverse shift (on gelu(y), C=512): y's channels are produced in 9 groups of
   57 padded to 64 partitions (576 rows = 4.5 blocks; host-permuted fc1_w
   columns), so each group starts at partition 0 or 64 (the HW requires
   compute-engine APs to start at 32-aligned partitions).  The gelu PSUM
   evacuation then writes each group directly into h at its inversely-shifted,
   edge-clipped token positions — the shift costs no extra passes.
   w / the gate / proj all use the same padded-576 channel layout (again via
   host-side weight permutation); padded rows are exactly zero throughout.

The only cross-core coupling is the global (T,H,W) mean feeding the softmax
gate.  It is done as TWO tiny AllReduces (one per frame): the first is
triggered halfway through the kernel and absorbs the cross-core launch skew
under frame-1 compute, so only the second's ~10us floor is exposed.

bf16 matmuls with f32 PSUM accumulation; output f32.  Frame 0's h/w branches
spill to DRAM (bf16) and stream back during the output phase to fit SBUF.
"""

import os
import sys

for _p in ("/opt/trn_rl_repo",):
    if os.path.isdir(_p) and _p not in sys.path:
        sys.path.append(_p)

import numpy as np
import ml_dtypes

import concourse.bass as bass  # noqa: F401
import concourse.mybir as mybir
import concourse.tile as tile
from concourse import bacc
from concourse.bass_utils import run_bass_kernel_spmd

# ---------------------------------------------------------------- constants
SHIFTS = [(1, 1), (1, 0), (1, -1), (0, 1), (0, 0), (0, -1), (-1, 1), (-1, 0), (-1, -1)]
NG = 9
B, T, H, W, C = 2, 8, 56, 56, 512
HID = 1024
NCORES = 8
NF = (B * T) // NCORES          # frames per core = 2
HWTOK = H * W                   # 3136 tokens per frame
RP = W + 1                      # padded row pitch = 57
GUARD = RP + 1                  # 58 zero tokens on each end
FRPAD = RP * H                  # 3192
XHSPAN = GUARD + FRPAD + GUARD  # 3308
RG = 7                          # row groups per frame
RGR = H // RG                   # 8 rows per group
RGT = RGR * W                   # 448 valid tokens per row group
RGP = RGR * RP                  # 456 padded tokens per row group
GS_HID = 114                    # hid shift-group size (9*114 = 1026 >= 1024)
GS_C = 57                       # C shift-group size (9*57 = 513 >= 512)
GPAD = 64                       # C shift groups padded to 64 partitions
CP = NG * GPAD                  # 576 padded C rows
YCB = (CP + 127) // 128         # 5 row-blocks (last half-used)
CCB = C // 128                  # 4
HCB = HID // 128                # 8
MEAN_N = float(T * H * W)

F32 = mybir.dt.float32
BF16 = mybir.dt.bfloat16
BF16_NP = ml_dtypes.bfloat16

_CACHE = {}


def _c_groups():
    """(g, n_ch, real channel range) for the 9 C shift groups."""
    out = []
    for g in range(NG):
        c0 = GS_C * g
        c1 = min(GS_C * (g + 1), C)
        out.append((g, c1 - c0, c0, c1))
    return out


# ---------------------------------------------------------------- device kernel
def build_nc():
    nc = bacc.Bacc("TRN2", target_bir_lowering=False, debug=False, num_devices=NCORES)

    dp = nc.declare_dram_parameter
    xT = dp("xT", [NF, 128, CCB, HWTOK], BF16, isOutput=False)
    fcw = dp("fcw", [128, CCB, NG * 128], BF16, isOutput=False)
    fcb = dp("fcb", [128, NG], F32, isOutput=False)
    fc1w = dp("fc1w", [128, NG, CP], BF16, isOutput=False)
    fc1b = dp("fc1b", [128, YCB], F32, isOutput=False)
    fc2w = dp("fc2w", [128, NG, CP], BF16, isOutput=False)
    fc2b = dp("fc2b", [128, YCB], F32, isOutput=False)
    projw = dp("projw", [128, YCB, C], BF16, isOutput=False)
    projb = dp("projb", [128, C], F32, isOutput=False)
    rw1w = dp("rw1w", [128, YCB, 128], BF16, isOutput=False)
    rw1b = dp("rw1b", [128, 1], F32, isOutput=False)
    rw2w = dp("rw2w", [128, 2 * YCB * 128], BF16, isOutput=False)
    rw2b = dp("rw2b", [128, 2 * YCB], F32, isOutput=False)
    bmask = dp("bmask", [128, B], F32, isOutput=False)
    out_d = dp("out", [NF, HWTOK, C], F32, isOutput=True)

    # spill space for the w branch of each frame + collective bounce buffers
    wsp = [nc.dram_tensor(f"wsp{f}", [128, YCB, HWTOK], BF16) for f in range(NF)]
    ccin = [nc.dram_tensor(f"ccin{f}", [B, 128, YCB], F32) for f in range(NF)]
    ccout = [
        nc.dram_tensor(f"ccout{f}", [B, 128, YCB], F32, addr_space="Shared")
        for f in range(NF)
    ]

    AF = mybir.ActivationFunctionType
    ALU = mybir.AluOpType

    with tile.TileContext(nc, num_cores=NCORES) as tc:
        with (
            tc.tile_pool(name="singles", bufs=1) as singles,
            tc.tile_pool(name="xh_pool", bufs=1) as xh_pool,
            tc.tile_pool(name="h_pool", bufs=2) as h_pool,
            tc.tile_pool(name="w_pool", bufs=2) as w_pool,
            tc.tile_pool(name="xt_pool", bufs=2) as xt_pool,
            tc.tile_pool(name="ostage", bufs=3) as ostage,
            tc.tile_pool(name="dstream", bufs=3) as dstream,
            tc.tile_pool(name="small", bufs=1) as small,
            tc.tile_pool(name="mmpsum", bufs=8, space="PSUM") as mmpsum,
        ):
            # ---- load weights (resident for the whole kernel)
            def load(name, shape, dtype, src):
                t = singles.tile(shape, dtype, name=name)
                nc.sync.dma_start(out=t, in_=src[:])
                return t

            # only what frame-0's fc pass needs is loaded up front; the rest
            # loads while it runs (keeps the kernel head short)
            fcw_s = load("fcw_s", [128, CCB, NG * 128], BF16, fcw)
            fcb_s = load("fcb_s", [128, NG], F32, fcb)
            _rest = {}

            def load_rest():
                _rest["fc1w_s"] = load("fc1w_s", [128, NG, CP], BF16, fc1w)
                _rest["fc1b_s"] = load("fc1b_s", [128, YCB], F32, fc1b)
                _rest["fc2w_s"] = load("fc2w_s", [128, NG, CP], BF16, fc2w)
                _rest["fc2b_s"] = load("fc2b_s", [128, YCB], F32, fc2b)
                _rest["projw_s"] = load("projw_s", [128, YCB, C], BF16, projw)
                _rest["projb_s"] = load("projb_s", [128, C], F32, projb)
                _rest["rw1w_s"] = load("rw1w_s", [128, YCB, 128], BF16, rw1w)
                _rest["rw1b_s"] = load("rw1b_s", [128, 1], F32, rw1b)
                _rest["rw2w_s"] = load("rw2w_s", [128, 2 * YCB * 128], BF16, rw2w)
                _rest["rw2b_s"] = load("rw2b_s", [128, 2 * YCB], F32, rw2b)
                _rest["bmask_s"] = load("bmask_s", [128, B], F32, bmask)

            a0_s = singles.tile([128, YCB], F32)   # gate for the h branch

            # xh, padded token layout, persistent across frames.
            xh = xh_pool.tile([128, NG, XHSPAN], BF16)
            # zero guards + per-row pad column once; the body is fully
            # rewritten by every frame's fc pass.
            nc.vector.memset(xh[:, :, :GUARD], 0.0)
            nc.vector.memset(xh[:, :, GUARD + FRPAD:], 0.0)
            xh_rows = xh[:, :, GUARD:GUARD + FRPAD].rearrange(
                "p g (r c) -> p g r c", c=RP
            )
            nc.vector.memset(xh_rows[:, :, :, W:], 0.0)

            hw_tiles = []
            part_sums = []

            for f in range(NF):
                # ---------------- A: xh = gelu(x @ fc_w + fc_b), group-blocked
                for rg in range(RG):
                    xt_t = xt_pool.tile([128, CCB, RGT], BF16, tag="xt")
                    nc.sync.dma_start(
                        out=xt_t, in_=xT[f, :, :, rg * RGT:(rg + 1) * RGT]
                    )
                    for mb in range(NG):
                        ps = mmpsum.tile([128, 512], F32, tag="mm")
                        for k in range(CCB):
                            nc.tensor.matmul(
                                ps[:, :RGT],
                                lhsT=fcw_s[:, k, mb * 128:(mb + 1) * 128],
                                rhs=xt_t[:, k, :],
                                start=(k == 0),
                                stop=(k == CCB - 1),
                            )
                        dst = xh[
                            :, mb, GUARD + rg * RGP:GUARD + (rg + 1) * RGP
                        ].rearrange("p (r c) -> p r c", c=RP)[:, :, :W]
                        src = ps[:, :RGT].rearrange("p (r c) -> p r c", c=W)
                        nc.scalar.activation(
                            out=dst, in_=src, func=AF.Gelu,
                            bias=fcb_s[:, mb:mb + 1],
                        )

                if f == 0:
                    # frame-0 fc pass is in flight; now bring in the rest
                    load_rest()
                    fc1w_s = _rest["fc1w_s"]; fc1b_s = _rest["fc1b_s"]
                    fc2w_s = _rest["fc2w_s"]; fc2b_s = _rest["fc2b_s"]
                    projw_s = _rest["projw_s"]; projb_s = _rest["projb_s"]
                    rw1w_s = _rest["rw1w_s"]; rw1b_s = _rest["rw1b_s"]
                    rw2w_s = _rest["rw2w_s"]; rw2b_s = _rest["rw2b_s"]
                    bmask_s = _rest["bmask_s"]

                # ---------------- C: h = invshift(gelu(shift(xh) @ fc1_w + b))
                # y channels live in 9 groups of 57 padded to 64 partitions
                # (576 rows = YCB blocks); the inverse shift is applied by the
                # gelu evacuation writing each group at shifted positions.
                h_t = h_pool.tile([128, YCB, HWTOK], BF16, tag="h")
                nc.gpsimd.memset(h_t[:], 0.0)
                h4 = h_t.rearrange("p c (i j) -> p c i j", j=W)
                hsum_st = small.tile([128, YCB, RG], F32, tag=f"hsst{f}")
                nc.vector.memset(hsum_st[:], 0.0)
                wsum_st = small.tile([128, YCB, RG], F32, tag=f"wsst{f}")
                nc.vector.memset(wsum_st[:], 0.0)
                for rg in range(RG):
                    for mb in range(YCB):
                        M = min(128, CP - mb * 128)
                        ps = mmpsum.tile([128, 512], F32, tag="mm")
                        for g in range(NG):
                            off = -(SHIFTS[g][0] * RP + SHIFTS[g][1])
                            s0 = GUARD + rg * RGP + off
                            rhs2 = xh[:, g, s0:s0 + RGP].rearrange(
                                "p (r c) -> p r c", c=RP
                            )[:, :, :W]
                            nc.tensor.matmul(
                                ps[:M, :RGT],
                                lhsT=fc1w_s[:, g, mb * 128:mb * 128 + M],
                                rhs=rhs2,
                                start=(g == 0),
                                stop=(g == NG - 1),
                            )
                        ps3 = ps[:, :RGT].rearrange("p (r c) -> p r c", c=W)
                        # two 64-partition group-halves per block (block 4:
                        # only the lower half carries group 8)
                        for half in range(2):
                            q0 = mb * 128 + half * GPAD
                            g = q0 // GPAD
                            if g >= NG:
                                continue
                            nch = min(GS_C * (g + 1), C) - GS_C * g
                            sh, sw = SHIFTS[g]
                            # h(i',j') = gelu_y(i'+sh, j'+sw); this window
                            # holds gelu_y rows [8rg, 8rg+8)
                            i0 = max(0, 8 * rg - sh)
                            i1 = min(H, 8 * rg + 8 - sh)
                            j0, j1 = max(0, -sw), min(W, W - sw)
                            p0 = half * GPAD
                            nc.scalar.activation(
                                out=h4[p0:p0 + nch, mb, i0:i1, j0:j1],
                                in_=ps3[
                                    p0:p0 + nch,
                                    i0 + sh - 8 * rg:i1 + sh - 8 * rg,
                                    j0 + sw:j1 + sw,
                                ],
                                func=AF.Gelu,
                                bias=fc1b_s[p0:p0 + nch, mb:mb + 1],
                                accum_out=hsum_st[p0:p0 + nch, mb, rg:rg + 1],
                            )

                # ---------------- B: w = gelu(xh @ fc2_w + fc2_b), padded-576,
                # built per row-group and spilled to DRAM (streamed back in D)
                for rg in range(RG):
                    w_rg = w_pool.tile([128, YCB, RGT], BF16, tag="wrg")
                    if True:
                        nc.vector.memset(w_rg[GPAD:, YCB - 1, :], 0.0)
                    for mb in range(YCB):
                        M = min(128, CP - mb * 128)
                        ps = mmpsum.tile([128, 512], F32, tag="mm")
                        for g in range(NG):
                            s0 = GUARD + rg * RGP
                            rhs2 = xh[:, g, s0:s0 + RGP].rearrange(
                                "p (r c) -> p r c", c=RP
                            )[:, :, :W]
                            nc.tensor.matmul(
                                ps[:M, :RGT],
                                lhsT=fc2w_s[:, g, mb * 128:mb * 128 + M],
                                rhs=rhs2,
                                start=(g == 0),
                                stop=(g == NG - 1),
                            )
                        dst = w_rg[:M, mb, :].rearrange("p (r c) -> p r c", c=W)
                        srcp = ps[:M, :RGT].rearrange("p (r c) -> p r c", c=W)
                        nc.scalar.activation(
                            out=dst, in_=srcp, func=AF.Gelu,
                            bias=fc2b_s[:M, mb:mb + 1],
                            accum_out=wsum_st[:M, mb, rg:rg + 1],
                        )
                    nc.sync.dma_start(
                        out=wsp[f][:, :, rg * RGT:(rg + 1) * RGT], in_=w_rg[:]
                    )

                # ---------------- per-frame gate partial sum + AllReduce
                hs = small.tile([128, YCB], F32, tag=f"hs{f}")
                nc.vector.tensor_reduce(
                    out=hs, in_=hsum_st[:], axis=mybir.AxisListType.X, op=ALU.add
                )
                ws = small.tile([128, YCB], F32, tag=f"ws{f}")
                nc.vector.tensor_reduce(
                    out=ws, in_=wsum_st[:], axis=mybir.AxisListType.X, op=ALU.add
                )
                part = small.tile([128, YCB], F32, tag=f"part{f}")
                nc.vector.tensor_tensor(part, hs, ws, ALU.add)
                part_sums.append(part)
                # mask into the own-batch row and AllReduce; frame 0's
                # collective overlaps frame 1's compute (and absorbs the
                # cross-core launch skew).
                t0 = small.tile([128, YCB], F32, tag=f"cca{f}")
                nc.vector.tensor_scalar_mul(t0, part, bmask_s[:, 0:1])
                t1 = small.tile([128, YCB], F32, tag=f"ccb{f}")
                nc.vector.tensor_scalar_mul(t1, part, bmask_s[:, 1:2])
                nc.sync.dma_start(out=ccin[f][0], in_=t0)
                nc.sync.dma_start(out=ccin[f][1], in_=t1)
                nc.gpsimd.collective_compute(
                    "AllReduce",
                    ALU.add,
                    replica_groups=[list(range(NCORES))],
                    ins=[ccin[f][:]],
                    outs=[ccout[f][:]],
                )

                hw_tiles.append(h_t)

            # keep TensorE's activity monitor warm across the second
            # AllReduce's latency window (junk matmuls, results unread) —
            # otherwise the whole output phase runs at the 4/8 cold clock
            for wi in range(110):
                wp = mmpsum.tile([128, 512], F32, tag="mm", name=f"warm{wi}")
                nc.tensor.matmul(
                    wp[:, :512],
                    lhsT=fcw_s[:, 0, 0:128],
                    rhs=fcw_s[:, 1, 0:512],
                    start=True,
                    stop=True,
                )

            # ---------------- combine the two AllReduce results -> z
            acc = []
            for f in range(NF):
                za = small.tile([128, YCB], F32, tag=f"za{f}")
                nc.sync.dma_start(out=za, in_=ccout[f][0])
                zb = small.tile([128, YCB], F32, tag=f"zb{f}")
                nc.sync.dma_start(out=zb, in_=ccout[f][1])
                nc.vector.tensor_scalar_mul(za, za, bmask_s[:, 0:1])
                nc.vector.tensor_scalar_mul(zb, zb, bmask_s[:, 1:2])
                s = small.tile([128, YCB], F32, tag=f"zs{f}")
                nc.vector.tensor_tensor(s, za, zb, ALU.add)
                acc.append(s)
            zsum = small.tile([128, YCB], F32, tag="zsum")
            nc.vector.tensor_tensor(zsum, acc[0], acc[1], ALU.add)
            zbf = small.tile([128, YCB], BF16, tag="zbf")
            nc.vector.tensor_copy(out=zbf, in_=zsum)

            # ---------------- gate: a = softmax over the 2 streams
            # (1/MEAN_N is folded into rw1w on the host)
            psg = mmpsum.tile([128, 512], F32, tag="mm", name="psg")[:, :1]
            for k in range(YCB):
                nc.tensor.matmul(
                    psg,
                    lhsT=rw1w_s[:, k, :],
                    rhs=zbf[:, k:k + 1],
                    start=(k == 0),
                    stop=(k == YCB - 1),
                )
            gv = small.tile([128, 1], BF16, tag="gv")
            nc.scalar.activation(out=gv, in_=psg, func=AF.Gelu, bias=rw1b_s[:, 0:1])
            psu = mmpsum.tile([128, 512], F32, tag="mm", name="psu")[:, :2 * YCB]
            for m in range(2 * YCB):
                nc.tensor.matmul(
                    psu[:, m:m + 1],
                    lhsT=rw2w_s[:, m * 128:(m + 1) * 128],
                    rhs=gv,
                    start=True,
                    stop=True,
                )
            uv = small.tile([128, 2 * YCB], F32, tag="uv")
            nc.vector.tensor_tensor(uv, psu, rw2b_s, ALU.add)
            l0, l1 = uv[:, 0:YCB], uv[:, YCB:2 * YCB]
            mx = small.tile([128, YCB], F32, tag="mx")
            nc.vector.tensor_tensor(mx, l0, l1, ALU.max)
            d0 = small.tile([128, YCB], F32, tag="d0")
            nc.vector.tensor_tensor(d0, l0, mx, ALU.subtract)
            d1 = small.tile([128, YCB], F32, tag="d1")
            nc.vector.tensor_tensor(d1, l1, mx, ALU.subtract)
            e0 = small.tile([128, YCB], F32, tag="e0")
            nc.scalar.activation(out=e0, in_=d0, func=AF.Exp)
            e1 = small.tile([128, YCB], F32, tag="e1")
            nc.scalar.activation(out=e1, in_=d1, func=AF.Exp)
            esum = small.tile([128, YCB], F32, tag="esum")
            nc.vector.tensor_tensor(esum, e0, e1, ALU.add)
            rec = small.tile([128, YCB], F32, tag="rec")
            nc.vector.reciprocal(rec, esum)
            nc.vector.tensor_tensor(a0_s, e0, rec, ALU.mult)

            # ---------------- D: out = (a0*h + (1-a0)*w) @ proj_w + proj_b
            def proj_blocks(gated_ap, fidx, tbase, ntok):
                """gated_ap: [128, YCB, ntok] bf16 SBUF ap (padded-576)."""
                m0 = 0
                while m0 < ntok:
                    M = min(128, ntok - m0)
                    pp = mmpsum.tile([128, 512], F32, tag="mm")
                    for kb in range(YCB):
                        nc.tensor.matmul(
                            pp[:M, :C],
                            lhsT=gated_ap[:, kb, m0:m0 + M],
                            rhs=projw_s[:, kb, :],
                            start=(kb == 0),
                            stop=(kb == YCB - 1),
                        )
                    ot = ostage.tile([128, C], F32, tag="ot")
                    nc.vector.tensor_tensor(ot[:M], pp[:M, :C], projb_s[:M], ALU.add)
                    nc.sync.dma_start(
                        out=out_d[fidx, tbase + m0:tbase + m0 + M, :], in_=ot[:M]
                    )
                    m0 += M

            def gate_inplace(h_ap, w_ap):
                """h_ap <- a0*h + (1-a0)*w   (= w + a0*(h-w)), in place.
                Per row-block chains so proj passes can start early."""
                for kb in range(YCB):
                    nc.vector.tensor_tensor(
                        h_ap[:, kb], h_ap[:, kb], w_ap[:, kb], ALU.subtract
                    )
                    nc.scalar.activation(
                        out=h_ap[:, kb], in_=h_ap[:, kb],
                        func=AF.Copy, scale=a0_s[:, kb:kb + 1],
                    )
                    nc.vector.tensor_tensor(
                        h_ap[:, kb], h_ap[:, kb], w_ap[:, kb], ALU.add
                    )

            # h is resident for both frames; stream each frame's w back in
            # 512-token chunks, gate in place on the h slice, then project.
            for fidx in (1, 0):
                h_t = hw_tiles[fidx]
                ck0 = 0
                while ck0 < HWTOK:
                    CK = min(512, HWTOK - ck0)
                    wc = dstream.tile([128, YCB, 512], BF16, tag="wc")
                    nc.sync.dma_start(
                        out=wc[:, :, :CK], in_=wsp[fidx][:, :, ck0:ck0 + CK]
                    )
                    gate_inplace(h_t[:, :, ck0:ck0 + CK], wc[:, :, :CK])
                    proj_blocks(h_t[:, :, ck0:ck0 + CK], fidx, ck0, CK)
                    ck0 += CK

    nc.compile()
    return nc


# ---------------------------------------------------------------- host side
def _prep_weights(fc_w, fc_b, fc1_w, fc1_b, fc2_w, fc2_b,
                  rw1_w, rw1_b, rw2_w, rw2_b, proj_w, proj_b):
    f32 = np.float32

    # padded-576 C layout: padded row q = 64*g + s  <->  channel c = 57*g + s
    qof = np.full((CP,), -1, np.int64)
    for g, nch, c0, _ in _c_groups():
        qof[GPAD * g:GPAD * g + nch] = np.arange(c0, c0 + nch)
    qvalid = qof >= 0
    qidx = np.where(qvalid, np.maximum(qof, 0), 0)

    def cols_to_padded576(m):  # [R, C] -> [R, CP] with zero pad cols
        out = np.zeros((m.shape[0], CP), f32)
        out[:, qvalid] = m[:, qidx[qvalid]]
        return out

    def rows_to_padded576(m):  # [C, N] -> [CP, N] with zero pad rows
        out = np.zeros((CP, m.shape[1]), f32)
        out[qvalid] = m[qidx[qvalid]]
        return out

    def vec_to_padded576(v):
        out = np.zeros((CP,), f32)
        out[qvalid] = v[qidx[qvalid]]
        return out

    # fc: columns permuted into 9 HID-groups of 114 (112 for g=8), pad to 128
    fcwp = np.zeros((C, NG * 128), f32)
    fcbp = np.zeros((NG * 128,), f32)
    for g in range(NG):
        n = min(GS_HID * (g + 1), HID) - GS_HID * g
        fcwp[:, 128 * g:128 * g + n] = fc_w[:, GS_HID * g:GS_HID * g + n]
        fcbp[128 * g:128 * g + n] = fc_b[GS_HID * g:GS_HID * g + n]
    fcw_h = np.ascontiguousarray(
        fcwp.reshape(CCB, 128, NG * 128).transpose(1, 0, 2)
    ).astype(BF16_NP)
    fcb_h = np.ascontiguousarray(fcbp.reshape(NG, 128).T).astype(f32)

    def hid_rows_grouped(wm):  # [HID, CP] -> [128, NG, CP] padded group rows
        wp = np.zeros((NG * 128, wm.shape[1]), f32)
        for g in range(NG):
            n = min(GS_HID * (g + 1), HID) - GS_HID * g
            wp[128 * g:128 * g + n] = wm[GS_HID * g:GS_HID * g + n]
        return np.ascontiguousarray(
            wp.reshape(NG, 128, wm.shape[1]).transpose(1, 0, 2)
        ).astype(BF16_NP)

    fc1w_h = hid_rows_grouped(cols_to_padded576(fc1_w))
    fc2w_h = hid_rows_grouped(cols_to_padded576(fc2_w))

    fc1bp = vec_to_padded576(fc1_b)
    fc2bp = vec_to_padded576(fc2_b)
    padb = np.zeros((YCB * 128,), f32)
    fc1b_h = padb.copy(); fc1b_h[:CP] = fc1bp
    fc1b_h = np.ascontiguousarray(fc1b_h.reshape(YCB, 128).T).astype(f32)
    fc2b_h = padb.copy(); fc2b_h[:CP] = fc2bp
    fc2b_h = np.ascontiguousarray(fc2b_h.reshape(YCB, 128).T).astype(f32)

    # proj: rows in padded-576 layout (pad rows zero), cols plain C
    projwp = np.zeros((YCB * 128, C), f32)
    projwp[:CP] = rows_to_padded576(proj_w)
    projw_h = np.ascontiguousarray(
        projwp.reshape(YCB, 128, C).transpose(1, 0, 2)
    ).astype(BF16_NP)
    projb_h = np.ascontiguousarray(
        np.broadcast_to(proj_b[None, :], (128, C))
    ).astype(f32)

    # rw1: rows in padded-576 layout, scaled by 1/MEAN_N (folds the mean)
    rw1p = np.zeros((YCB * 128, C // 4), f32)
    rw1p[:CP] = rows_to_padded576(rw1_w / MEAN_N)
    rw1w_h = np.ascontiguousarray(
        rw1p.reshape(YCB, 128, C // 4).transpose(1, 0, 2)
    ).astype(BF16_NP)
    rw1b_h = np.ascontiguousarray(rw1_b[:, None]).astype(f32)

    # rw2 columns: stream-0 logits in padded cols [0, CP), stream-1 logits in
    # padded cols [YCB*128, YCB*128 + CP) — so the device's 128-wide M-blocks
    # 0..4 are stream 0 and 5..9 are stream 1.
    NQ = YCB * 128
    rw2p = np.zeros((128, 2 * NQ), f32)
    rw2p[:, 0:CP][:, qvalid] = rw2_w[:, 2 * qidx[qvalid]]
    rw2p[:, NQ:NQ + CP][:, qvalid] = rw2_w[:, 2 * qidx[qvalid] + 1]
    rw2w_h = np.ascontiguousarray(rw2p).astype(BF16_NP)
    rw2b_full = np.zeros((2 * NQ,), f32)
    rw2b_full[0:CP][qvalid] = rw2_b[2 * qidx[qvalid]]
    rw2b_full[NQ:NQ + CP][qvalid] = rw2_b[2 * qidx[qvalid] + 1]
    rw2b_h = np.ascontiguousarray(rw2b_full.reshape(2 * YCB, 128).T).astype(f32)

    return dict(
        fcw=fcw_h, fcb=fcb_h, fc1w=fc1w_h, fc1b=fc1b_h, fc2w=fc2w_h,
        fc2b=fc2b_h, projw=projw_h, projb=projb_h, rw1w=rw1w_h, rw1b=rw1b_h,
        rw2w=rw2w_h, rw2b=rw2b_h,
    )


def _get_nc():
    if "nc" not in _CACHE:
        _CACHE["nc"] = build_nc()
    return _CACHE["nc"]


def run(inputs, trace=False, trace_kwargs=None):
    """Run the SPMD kernel; returns (full_output, BassKernelResults)."""
    x = np.asarray(inputs["x"], np.float32)
    shared = _prep_weights(
        np.asarray(inputs["fc_w"], np.float32), np.asarray(inputs["fc_b"], np.float32),
        np.asarray(inputs["fc1_w"], np.float32), np.asarray(inputs["fc1_b"], np.float32),
        np.asarray(inputs["fc2_w"], np.float32), np.asarray(inputs["fc2_b"], np.float32),
        np.asarray(inputs["rw1_w"], np.float32), np.asarray(inputs["rw1_b"], np.float32),
        np.asarray(inputs["rw2_w"], np.float32), np.asarray(inputs["rw2_b"], np.float32),
        np.asarray(inputs["proj_w"], np.float32), np.asarray(inputs["proj_b"], np.float32),
    )

    xf = x.reshape(B * T, HWTOK, C)
    in_maps = []
    for c in range(NCORES):
        sh = xf[NF * c:NF * (c + 1)]                      # [NF, 3136, 512]
        xt = sh.transpose(0, 2, 1).reshape(NF, CCB, 128, HWTOK)
        xt = np.ascontiguousarray(xt.transpose(0, 2, 1, 3)).astype(BF16_NP)
        bm = np.zeros((128, B), np.float32)
        bm[:, (NF * c) // T] = 1.0
        m = dict(shared)
        m["xT"] = xt
        m["bmask"] = bm
        in_maps.append(m)

    nc = _get_nc()
    res = run_bass_kernel_spmd(
        nc, in_maps, list(range(NCORES)),
        trace=trace, **(dict(trace_kwargs=trace_kwargs) if trace_kwargs else {}),
    )

    out = np.empty((B * T, HWTOK, C), np.float32)
    for c in range(NCORES):
        out[NF * c:NF * (c + 1)] = res.results[c]["out"]
    return out.reshape(B, T, H, W, C), res


def kernel(**inputs) -> np.ndarray:
    full, _ = run(inputs, trace=False)
    return full



# revision 9
# speedup vs baseline: 1.0097x; 1.0097x over previous
"""Trainium2 Bass kernel for nn_Mlp_cnn_shift (dense CNN MLP with 3x3 patch-shift
and a softmax-gated mix of two branches).

Strategy
--------
Data-parallel over the 16 (B,T) frames: each of the 8 NeuronCores processes 2
frames end-to-end.  All activations are kept channel-major ([C, tokens]) so the
channel contraction of every matmul has K on partitions, and `x` is
pre-transposed/cast on the host so no on-device transpose is needed.

Patch-shift handling:
 * forward shift (on xh, HID=1024): xh is stored in a zero-padded token layout
   (row pitch 57 = 56 cols + 1 zero pad col, 58-token zero guards per frame)
   and in 9 channel groups of 114 padded to 128 partitions each (host-permuted
   fc_w columns / fc1_w+fc2_w rows).  Every (dh,dw) roll then becomes a pure
   token offset in the fc1 matmul's rhs access pattern, with the zero padding
   reproducing the reference's zero-fill boundary exactly.
 * inverse shift (on gelu(y), C=512): y's channels are produced in 9 groups of
   57 padded to 64 partitions (576 rows = 4.5 blocks; host-permuted fc1_w
   columns), so each group starts at partition 0 or 64 (the HW requires
   compute-engine APs to start at 32-aligned partitions).  The gelu PSUM
   evacuation then writes each group directly into h at its inversely-shifted,
   edge-clipped token positions — the shift costs no extra passes.
   w / the gate / proj all use the same padded-576 channel layout (again via
   host-side weight permutation); padded rows are exactly zero throughout.

The half-used 5th channel block (group 8 alone, M=64) of fc1 and fc2 is
computed as a column-tiled PAIR: fc1's g8 output on PE col-tile (0,0) ->
psum partitions 0:64, fc2's g8 on col-tile (0,64) -> psum 64:128, both
streaming their own rhs concurrently, so the pair costs one matmul slot
instead of two.  fc2's half lands on upper partitions and is moved to the
canonical lower half by a tiny SBUF->SBUF DMA; the gate partial sums keep the
two halves in separate columns (6-column payload) so no partition shuffle is
needed on the reduction path.

Instead of w, the kernel spills d = h - w (computed per row-group on the DVE,
one row-group behind the evacuations); the post-gate combine is then just
gated = h + (a0-1)*d, two cheap DVE ops per chunk.

The only cross-core coupling is the global (T,H,W) mean feeding the softmax
gate, done as per-batch subgroup AllReduces ([0-3] and [4-7]), one per frame:
the first overlaps frame-1 compute, only the second's latency is exposed and
is bridged by a short budget of warm matmuls (keeps the PE activity monitor
from re-throttling the clock).

bf16 matmuls with f32 PSUM accumulation; output f32.
"""

import os
import sys

for _p in ("/opt/trn_rl_repo",):
    if os.path.isdir(_p) and _p not in sys.path:
        sys.path.append(_p)

import numpy as np
import ml_dtypes

import concourse.bass as bass  # noqa: F401
import concourse.mybir as mybir
import concourse.tile as tile
from concourse import bacc
from concourse.bass_utils import run_bass_kernel_spmd

# ---------------------------------------------------------------- constants
SHIFTS = [(1, 1), (1, 0), (1, -1), (0, 1), (0, 0), (0, -1), (-1, 1), (-1, 0), (-1, -1)]
NG = 9
B, T, H, W, C = 2, 8, 56, 56, 512
HID = 1024
NCORES = 8
NF = (B * T) // NCORES          # frames per core = 2
HWTOK = H * W                   # 3136 tokens per frame
RP = W + 1                      # padded row pitch = 57
GUARD = RP + 1                  # 58 zero tokens on each end
FRPAD = RP * H                  # 3192
XHSPAN = GUARD + FRPAD + GUARD  # 3308
RG = 7                          # row groups per frame
RGR = H // RG                   # 8 rows per group
RGT = RGR * W                   # 448 valid tokens per row group
RGP = RGR * RP                  # 456 padded tokens per row group
GS_HID = 114                    # hid shift-group size (9*114 = 1026 >= 1024)
GS_C = 57                       # C shift-group size (9*57 = 513 >= 512)
GPAD = 64                       # C shift groups padded to 64 partitions
CP = NG * GPAD                  # 576 padded C rows
YCB = (CP + 127) // 128         # 5 row-blocks (last half-used)
ZC = YCB + 1                    # gate-payload columns (blk4 h/w kept separate)
CCB = C // 128                  # 4
HCB = HID // 128                # 8
MEAN_N = float(T * H * W)
WARM_MMS = 52                   # AllReduce-window bridge matmuls
HEAD_MMS = 8                    # kernel-head HAM warmup matmuls

F32 = mybir.dt.float32
BF16 = mybir.dt.bfloat16
BF16_NP = ml_dtypes.bfloat16

_CACHE = {}


def _c_groups():
    """(g, n_ch, real channel range) for the 9 C shift groups."""
    out = []
    for g in range(NG):
        c0 = GS_C * g
        c1 = min(GS_C * (g + 1), C)
        out.append((g, c1 - c0, c0, c1))
    return out


# ---------------------------------------------------------------- device kernel
def build_nc():
    nc = bacc.Bacc("TRN2", target_bir_lowering=False, debug=False, num_devices=NCORES)

    dp = nc.declare_dram_parameter
    xT = dp("xT", [NF, 128, CCB, HWTOK], BF16, isOutput=False)
    fcw = dp("fcw", [128, CCB, NG * 128], BF16, isOutput=False)
    fcb = dp("fcb", [128, NG], F32, isOutput=False)
    fc1w = dp("fc1w", [128, NG, CP], BF16, isOutput=False)
    fc1b = dp("fc1b", [128, YCB], F32, isOutput=False)
    fc2w = dp("fc2w", [128, NG, CP], BF16, isOutput=False)
    fc2b = dp("fc2b", [128, YCB], F32, isOutput=False)
    projw = dp("projw", [128, YCB, C], BF16, isOutput=False)
    projb = dp("projb", [128, C], F32, isOutput=False)
    rw1w = dp("rw1w", [128, ZC, 128], BF16, isOutput=False)
    rw1b = dp("rw1b", [128, 1], F32, isOutput=False)
    rw2w = dp("rw2w", [128, 2 * YCB * 128], BF16, isOutput=False)
    rw2b = dp("rw2b", [128, 2 * YCB], F32, isOutput=False)
    out_d = dp("out", [NF, HWTOK, C], F32, isOutput=True)

    # spill space for d = h - w of each frame + collective bounce buffers
    dsp = [nc.dram_tensor(f"dsp{f}", [128, YCB, HWTOK], BF16) for f in range(NF)]
    ccin = [nc.dram_tensor(f"ccin{f}", [128, ZC], F32) for f in range(NF)]
    ccout = [nc.dram_tensor(f"ccout{f}", [128, ZC], F32) for f in range(NF)]

    AF = mybir.ActivationFunctionType
    ALU = mybir.AluOpType
    GROUPS = [list(range(NCORES // 2)), list(range(NCORES // 2, NCORES))]

    with tile.TileContext(nc, num_cores=NCORES) as tc:
        with (
            tc.tile_pool(name="singles", bufs=1) as singles,
            tc.tile_pool(name="xh_pool", bufs=1) as xh_pool,
            tc.tile_pool(name="h_pool", bufs=2) as h_pool,
            tc.tile_pool(name="w_pool", bufs=3) as w_pool,
            tc.tile_pool(name="xt_pool", bufs=2) as xt_pool,
            tc.tile_pool(name="ostage", bufs=2) as ostage,
            tc.tile_pool(name="dstream", bufs=3) as dstream,
            tc.tile_pool(name="small", bufs=1) as small,
            tc.tile_pool(name="mmpsum", bufs=8, space="PSUM") as mmpsum,
        ):
            # ---- HAM warm-up: junk matmuls on a memset tile, no DMA deps,
            # so the PE is already at full clock when the first real matmul
            # arrives (~3us in, once the first weight/x slices land).
            jt = singles.tile([128, 640], BF16, name="jt")
            nc.vector.memset(jt[:], 0.0)
            for wi in range(HEAD_MMS):
                wp = mmpsum.tile([128, 512], F32, tag="mm", name=f"hw{wi}")
                nc.tensor.matmul(
                    wp[:, :512], lhsT=jt[:, 0:128], rhs=jt[:, 128:640],
                    start=True, stop=True,
                )

            # ---- load weights (resident for the whole kernel)
            def load(name, shape, dtype, src):
                t = singles.tile(shape, dtype, name=name)
                nc.sync.dma_start(out=t, in_=src[:])
                return t

            # fc weights load per k-slice so the first matmul can start after
            # ~1/4 of the transfer; the rest of the weights load while frame
            # 0's fc pass runs.
            fcw_s = singles.tile([128, CCB, NG * 128], BF16, name="fcw_s")
            for k in range(CCB):
                nc.sync.dma_start(out=fcw_s[:, k, :], in_=fcw[:, k, :])
            fcb_s = load("fcb_s", [128, NG], F32, fcb)
            _rest = {}

            def load_rest():
                _rest["fc1w_s"] = load("fc1w_s", [128, NG, CP], BF16, fc1w)
                _rest["fc1b_s"] = load("fc1b_s", [128, YCB], F32, fc1b)
                _rest["fc2w_s"] = load("fc2w_s", [128, NG, CP], BF16, fc2w)
                _rest["fc2b_s"] = load("fc2b_s", [128, YCB], F32, fc2b)
                _rest["projw_s"] = load("projw_s", [128, YCB, C], BF16, projw)
                _rest["projb_s"] = load("projb_s", [128, C], F32, projb)
                _rest["rw1w_s"] = load("rw1w_s", [128, ZC, 128], BF16, rw1w)
                _rest["rw1b_s"] = load("rw1b_s", [128, 1], F32, rw1b)
                _rest["rw2w_s"] = load("rw2w_s", [128, 2 * YCB * 128], BF16, rw2w)
                _rest["rw2b_s"] = load("rw2b_s", [128, 2 * YCB], F32, rw2b)

            a0_s = singles.tile([128, YCB], F32)   # gate for the h branch
            a1_s = singles.tile([128, YCB], F32)   # a0 - 1


            # xh, padded token layout, persistent across frames.
            xh = xh_pool.tile([128, NG, XHSPAN], BF16)
            nc.vector.memset(xh[:, :, :GUARD], 0.0)
            nc.vector.memset(xh[:, :, GUARD + FRPAD:], 0.0)
            xh_rows = xh[:, :, GUARD:GUARD + FRPAD].rearrange(
                "p g (r c) -> p g r c", c=RP
            )
            nc.vector.memset(xh_rows[:, :, :, W:], 0.0)

            hw_tiles = []

            def plain_rhs(rg):
                s0 = GUARD + rg * RGP
                return xh[:, :, s0:s0 + RGP]

            for f in range(NF):
                # ---------------- A: xh = gelu(x @ fc_w + fc_b), group-blocked
                for rg in range(RG):
                    xt_t = xt_pool.tile([128, CCB, RGT], BF16, tag="xt")
                    nc.sync.dma_start(
                        out=xt_t, in_=xT[f, :, :, rg * RGT:(rg + 1) * RGT]
                    )
                    for mb in range(NG):
                        ps = mmpsum.tile([128, 512], F32, tag="mm")
                        for k in range(CCB):
                            nc.tensor.matmul(
                                ps[:, :RGT],
                                lhsT=fcw_s[:, k, mb * 128:(mb + 1) * 128],
                                rhs=xt_t[:, k, :],
                                start=(k == 0),
                                stop=(k == CCB - 1),
                            )
                        dst = xh[
                            :, mb, GUARD + rg * RGP:GUARD + (rg + 1) * RGP
                        ].rearrange("p (r c) -> p r c", c=RP)[:, :, :W]
                        src = ps[:, :RGT].rearrange("p (r c) -> p r c", c=W)
                        nc.scalar.activation(
                            out=dst, in_=src, func=AF.Gelu,
                            bias=fcb_s[:, mb:mb + 1],
                        )

                if f == 0:
                    # frame-0 fc pass is in flight; now bring in the rest
                    load_rest()
                    fc1w_s = _rest["fc1w_s"]; fc1b_s = _rest["fc1b_s"]
                    fc2w_s = _rest["fc2w_s"]; fc2b_s = _rest["fc2b_s"]
                    projw_s = _rest["projw_s"]; projb_s = _rest["projb_s"]
                    rw1w_s = _rest["rw1w_s"]; rw1b_s = _rest["rw1b_s"]
                    rw2w_s = _rest["rw2w_s"]; rw2b_s = _rest["rw2b_s"]

                # ---------------- C: h = invshift(gelu(shift(xh) @ fc1_w + b))
                # full 128-blocks 0..3 here; block 4 (g8, M=64) is fused with
                # fc2's block 4 as a column-tiled pair in the B pass below.
                h_t = h_pool.tile([128, YCB, HWTOK], BF16, tag="h")
                nc.gpsimd.memset(h_t[:], 0.0)
                h4 = h_t.rearrange("p c (i j) -> p c i j", j=W)
                hsum_st = small.tile([128, YCB, RG], F32, tag=f"hsst{f}")
                nc.vector.memset(hsum_st[:], 0.0)
                wsum_st = small.tile([128, YCB, RG], F32, tag=f"wsst{f}")
                nc.vector.memset(wsum_st[:], 0.0)

                def h_evac(ps, rg, mb, halves):
                    ps3 = ps[:, :RGT].rearrange("p (r c) -> p r c", c=W)
                    for half in halves:
                        q0 = mb * 128 + half * GPAD
                        g = q0 // GPAD
                        nch = min(GS_C * (g + 1), C) - GS_C * g
                        sh, sw = SHIFTS[g]
                        i0 = max(0, 8 * rg - sh)
                        i1 = min(H, 8 * rg + 8 - sh)
                        j0, j1 = max(0, -sw), min(W, W - sw)
                        p0 = half * GPAD
                        nc.scalar.activation(
                            out=h4[p0:p0 + nch, mb, i0:i1, j0:j1],
                            in_=ps3[
                                p0:p0 + nch,
                                i0 + sh - 8 * rg:i1 + sh - 8 * rg,
                                j0 + sw:j1 + sw,
                            ],
                            func=AF.Gelu,
                            bias=fc1b_s[p0:p0 + nch, mb:mb + 1],
                            accum_out=hsum_st[p0:p0 + nch, mb, rg:rg + 1],
                        )

                for rg in range(RG):
                    for mb in range(YCB - 1):
                        ps = mmpsum.tile([128, 512], F32, tag="mm")
                        for g in range(NG):
                            off = -(SHIFTS[g][0] * RP + SHIFTS[g][1])
                            s0 = GUARD + rg * RGP + off
                            rhs2 = xh[:, g, s0:s0 + RGP].rearrange(
                                "p (r c) -> p r c", c=RP
                            )[:, :, :W]
                            nc.tensor.matmul(
                                ps[:, :RGT],
                                lhsT=fc1w_s[:, g, mb * 128:(mb + 1) * 128],
                                rhs=rhs2,
                                start=(g == 0),
                                stop=(g == NG - 1),
                            )
                        h_evac(ps, rg, mb, (0, 1))

                # ---------------- B: w = gelu(xh @ fc2_w + fc2_b) blocks 0..3
                # + the (fc1 g8 | fc2 g8) column-tiled pair, + d = h - w one
                # row-group behind (spilled to DRAM for the output phase).
                prev_w = [None] * RG

                def d_spill(rg):
                    w_prev = prev_w[rg]
                    nc.vector.tensor_tensor(
                        w_prev[:],
                        h_t[:, :, rg * RGT:(rg + 1) * RGT],
                        w_prev[:],
                        ALU.subtract,
                    )
                    nc.sync.dma_start(
                        out=dsp[f][:, :, rg * RGT:(rg + 1) * RGT], in_=w_prev[:]
                    )

                for rg in range(RG):
                    w_rg = w_pool.tile([128, YCB, RGT], BF16, tag="wrg")
                    rhs_pl = plain_rhs(rg).rearrange(
                        "p g (r c) -> p g r c", c=RP
                    )[:, :, :, :W]
                    for mb in range(YCB - 1):
                        ps = mmpsum.tile([128, 512], F32, tag="mm")
                        for g in range(NG):
                            nc.tensor.matmul(
                                ps[:, :RGT],
                                lhsT=fc2w_s[:, g, mb * 128:(mb + 1) * 128],
                                rhs=rhs_pl[:, g],
                                start=(g == 0),
                                stop=(g == NG - 1),
                            )
                        dst = w_rg[:, mb, :].rearrange("p (r c) -> p r c", c=W)
                        srcp = ps[:, :RGT].rearrange("p (r c) -> p r c", c=W)
                        nc.scalar.activation(
                            out=dst, in_=srcp, func=AF.Gelu,
                            bias=fc2b_s[:, mb:mb + 1],
                            accum_out=wsum_st[:, mb, rg:rg + 1],
                        )

                    # block-4 pair: fc1 g8 -> col-tile (0,0) psum[0:64],
                    # fc2 g8 -> col-tile (0,64) psum[64:128], concurrent.
                    mb = YCB - 1
                    ps = mmpsum.tile([128, 512], F32, tag="mm")
                    for g in range(NG):
                        off = -(SHIFTS[g][0] * RP + SHIFTS[g][1])
                        s0 = GUARD + rg * RGP + off
                        rhs_sh = xh[:, g, s0:s0 + RGP].rearrange(
                            "p (r c) -> p r c", c=RP
                        )[:, :, :W]
                        nc.tensor.matmul(
                            ps[0:64, :RGT],
                            lhsT=fc1w_s[:, g, mb * 128:mb * 128 + 64],
                            rhs=rhs_sh,
                            start=(g == 0),
                            stop=(g == NG - 1),
                            tile_position=(0, 0),
                        )
                        nc.tensor.matmul(
                            ps[64:128, :RGT],
                            lhsT=fc2w_s[:, g, mb * 128:mb * 128 + 64],
                            rhs=rhs_pl[:, g],
                            start=(g == 0),
                            stop=(g == NG - 1),
                            tile_position=(0, 64),
                        )
                    # h half (psum 0:64) exactly as the normal evacuation
                    h_evac(ps, rg, mb, (0,))
                    # w half (psum 64:128) -> upper block-4 rows, then a tiny
                    # SBUF->SBUF DMA moves it to the canonical lower half
                    dstw = w_rg[64:128, mb, :].rearrange("p (r c) -> p r c", c=W)
                    srcw = ps[64:128, :RGT].rearrange("p (r c) -> p r c", c=W)
                    nc.scalar.activation(
                        out=dstw, in_=srcw, func=AF.Gelu,
                        bias=fc2b_s[64:128, mb:mb + 1],
                        accum_out=wsum_st[64:128, mb, rg:rg + 1],
                    )
                    nc.sync.dma_start(
                        out=w_rg[0:64, mb, :], in_=w_rg[64:128, mb, :]
                    )

                    prev_w[rg] = w_rg
                    if rg >= 1:
                        d_spill(rg - 1)
                d_spill(RG - 1)

                # ---------------- per-frame gate partial sums + AllReduce.
                # blocks 0..3 carry h+w summed; block 4's h (lower partitions)
                # and w (upper partitions) ride in separate columns 4 and 5.
                hs = small.tile([128, YCB], F32, tag=f"hs{f}")
                nc.vector.tensor_reduce(
                    out=hs, in_=hsum_st[:], axis=mybir.AxisListType.X, op=ALU.add
                )
                ws = small.tile([128, YCB], F32, tag=f"ws{f}")
                nc.vector.tensor_reduce(
                    out=ws, in_=wsum_st[:], axis=mybir.AxisListType.X, op=ALU.add
                )
                part = small.tile([128, ZC], F32, tag=f"part{f}")
                nc.vector.tensor_tensor(
                    part[:, 0:YCB - 1], hs[:, 0:YCB - 1], ws[:, 0:YCB - 1],
                    ALU.add,
                )
                nc.vector.tensor_copy(
                    out=part[:, YCB - 1:YCB], in_=hs[:, YCB - 1:YCB]
                )
                nc.vector.tensor_copy(
                    out=part[:, YCB:ZC], in_=ws[:, YCB - 1:YCB]
                )
                nc.sync.dma_start(out=ccin[f][:], in_=part)
                nc.gpsimd.collective_compute(
                    "AllReduce",
                    ALU.add,
                    replica_groups=GROUPS,
                    ins=[ccin[f][:]],
                    outs=[ccout[f][:]],
                )

                hw_tiles.append(h_t)

            # bridge the second AllReduce's latency window with junk matmuls
            # (results unread) so the PE activity monitor keeps the full
            # clock; sized to the typical window, not generously.
            for wi in range(WARM_MMS):
                wp = mmpsum.tile([128, 512], F32, tag="mm", name=f"warm{wi}")
                nc.tensor.matmul(
                    wp[:, :512],
                    lhsT=fcw_s[:, 0, 0:128],
                    rhs=fcw_s[:, 1, 0:512],
                    start=True,
                    stop=True,
                )

            # ---------------- combine the two AllReduce results -> z
            acc = []
            for f in range(NF):
                za = small.tile([128, ZC], F32, tag=f"za{f}")
                nc.sync.dma_start(out=za, in_=ccout[f][:])
                acc.append(za)
            zsum = small.tile([128, ZC], F32, tag="zsum")
            nc.vector.tensor_tensor(zsum, acc[0], acc[1], ALU.add)
            zbf = small.tile([128, ZC], BF16, tag="zbf")
            nc.vector.tensor_copy(out=zbf, in_=zsum)

            # ---------------- gate: a = softmax over the 2 streams
            # (1/MEAN_N is folded into rw1w on the host)
            psg = mmpsum.tile([128, 512], F32, tag="mm", name="psg")[:, :1]
            for k in range(ZC):
                nc.tensor.matmul(
                    psg,
                    lhsT=rw1w_s[:, k, :],
                    rhs=zbf[:, k:k + 1],
                    start=(k == 0),
                    stop=(k == ZC - 1),
                )
            gv = small.tile([128, 1], BF16, tag="gv")
            nc.scalar.activation(out=gv, in_=psg, func=AF.Gelu, bias=rw1b_s[:, 0:1])
            psu = mmpsum.tile([128, 512], F32, tag="mm", name="psu")[:, :2 * YCB]
            for m in range(2 * YCB):
                nc.tensor.matmul(
                    psu[:, m:m + 1],
                    lhsT=rw2w_s[:, m * 128:(m + 1) * 128],
                    rhs=gv,
                    start=True,
                    stop=True,
                )
            uv = small.tile([128, 2 * YCB], F32, tag="uv")
            nc.vector.tensor_tensor(uv, psu, rw2b_s, ALU.add)
            l0, l1 = uv[:, 0:YCB], uv[:, YCB:2 * YCB]
            mx = small.tile([128, YCB], F32, tag="mx")
            nc.vector.tensor_tensor(mx, l0, l1, ALU.max)
            d0 = small.tile([128, YCB], F32, tag="d0")
            nc.vector.tensor_tensor(d0, l0, mx, ALU.subtract)
            d1 = small.tile([128, YCB], F32, tag="d1")
            nc.vector.tensor_tensor(d1, l1, mx, ALU.subtract)
            e0 = small.tile([128, YCB], F32, tag="e0")
            nc.scalar.activation(out=e0, in_=d0, func=AF.Exp)
            e1 = small.tile([128, YCB], F32, tag="e1")
            nc.scalar.activation(out=e1, in_=d1, func=AF.Exp)
            esum = small.tile([128, YCB], F32, tag="esum")
            nc.vector.tensor_tensor(esum, e0, e1, ALU.add)
            rec = small.tile([128, YCB], F32, tag="rec")
            nc.vector.reciprocal(rec, esum)
            nc.vector.tensor_tensor(a0_s, e0, rec, ALU.mult)
            nc.vector.tensor_scalar_sub(a1_s, a0_s, 1.0)

            # ---------------- D: out = (h + (a0-1)*d) @ proj_w + proj_b
            def proj_blocks(gated_ap, fidx, tbase, ntok):
                """gated_ap: [128, YCB, ntok] bf16 SBUF ap (padded-576)."""
                m0 = 0
                while m0 < ntok:
                    M = min(128, ntok - m0)
                    pp = mmpsum.tile([128, 512], F32, tag="mm")
                    for kb in range(YCB):
                        nc.tensor.matmul(
                            pp[:M, :C],
                            lhsT=gated_ap[:, kb, m0:m0 + M],
                            rhs=projw_s[:, kb, :],
                            start=(kb == 0),
                            stop=(kb == YCB - 1),
                        )
                    ot = ostage.tile([128, C], F32, tag="ot")
                    nc.vector.tensor_tensor(ot[:M], pp[:M, :C], projb_s[:M], ALU.add)
                    nc.sync.dma_start(
                        out=out_d[fidx, tbase + m0:tbase + m0 + M, :], in_=ot[:M]
                    )
                    m0 += M

            # h is resident for both frames; stream each frame's d back in
            # 512-token chunks, gate into the d tile, then project.
            for fidx in (1, 0):
                h_t = hw_tiles[fidx]
                ck0 = 0
                while ck0 < HWTOK:
                    CK = min(512, HWTOK - ck0)
                    dc = dstream.tile([128, YCB, 512], BF16, tag="wc")
                    nc.sync.dma_start(
                        out=dc[:, :, :CK], in_=dsp[fidx][:, :, ck0:ck0 + CK]
                    )
                    for kb in range(YCB):
                        nc.vector.tensor_scalar_mul(
                            dc[:, kb, :CK], dc[:, kb, :CK], a1_s[:, kb:kb + 1]
                        )
                        nc.vector.tensor_tensor(
                            dc[:, kb, :CK],
                            h_t[:, kb, ck0:ck0 + CK],
                            dc[:, kb, :CK],
                            ALU.add,
                        )
                    proj_blocks(dc[:, :, :CK], fidx, ck0, CK)
                    ck0 += CK

    nc.compile()
    return nc


# ---------------------------------------------------------------- host side
def _prep_weights(fc_w, fc_b, fc1_w, fc1_b, fc2_w, fc2_b,
                  rw1_w, rw1_b, rw2_w, rw2_b, proj_w, proj_b):
    f32 = np.float32

    # padded-576 C layout: padded row q = 64*g + s  <->  channel c = 57*g + s
    qof = np.full((CP,), -1, np.int64)
    for g, nch, c0, _ in _c_groups():
        qof[GPAD * g:GPAD * g + nch] = np.arange(c0, c0 + nch)
    qvalid = qof >= 0
    qidx = np.where(qvalid, np.maximum(qof, 0), 0)

    def cols_to_padded576(m):  # [R, C] -> [R, CP] with zero pad cols
        out = np.zeros((m.shape[0], CP), f32)
        out[:, qvalid] = m[:, qidx[qvalid]]
        return out

    def rows_to_padded576(m):  # [C, N] -> [CP, N] with zero pad rows
        out = np.zeros((CP, m.shape[1]), f32)
        out[qvalid] = m[qidx[qvalid]]
        return out

    def vec_to_padded576(v):
        out = np.zeros((CP,), f32)
        out[qvalid] = v[qidx[qvalid]]
        return out

    # fc: columns permuted into 9 HID-groups of 114 (112 for g=8), pad to 128
    fcwp = np.zeros((C, NG * 128), f32)
    fcbp = np.zeros((NG * 128,), f32)
    for g in range(NG):
        n = min(GS_HID * (g + 1), HID) - GS_HID * g
        fcwp[:, 128 * g:128 * g + n] = fc_w[:, GS_HID * g:GS_HID * g + n]
        fcbp[128 * g:128 * g + n] = fc_b[GS_HID * g:GS_HID * g + n]
    fcw_h = np.ascontiguousarray(
        fcwp.reshape(CCB, 128, NG * 128).transpose(1, 0, 2)
    ).astype(BF16_NP)
    fcb_h = np.ascontiguousarray(fcbp.reshape(NG, 128).T).astype(f32)

    def hid_rows_grouped(wm):  # [HID, CP] -> [128, NG, CP] padded group rows
        wp = np.zeros((NG * 128, wm.shape[1]), f32)
        for g in range(NG):
            n = min(GS_HID * (g + 1), HID) - GS_HID * g
            wp[128 * g:128 * g + n] = wm[GS_HID * g:GS_HID * g + n]
        return np.ascontiguousarray(
            wp.reshape(NG, 128, wm.shape[1]).transpose(1, 0, 2)
        ).astype(BF16_NP)

    fc1w_h = hid_rows_grouped(cols_to_padded576(fc1_w))
    fc2w_h = hid_rows_grouped(cols_to_padded576(fc2_w))

    fc1bp = vec_to_padded576(fc1_b)
    fc2bp = vec_to_padded576(fc2_b)
    padb = np.zeros((YCB * 128,), f32)
    fc1b_h = padb.copy(); fc1b_h[:CP] = fc1bp
    fc1b_h = np.ascontiguousarray(fc1b_h.reshape(YCB, 128).T).astype(f32)
    # fc2 biases: block 4's g8 biases duplicated on the upper 64 partitions,
    # where the column-tiled pair's w half is evacuated.
    fc2b_h = padb.copy(); fc2b_h[:CP] = fc2bp
    g8n = C - GS_C * (NG - 1)
    fc2b_h[4 * 128 + 64:4 * 128 + 64 + g8n] = fc2b_h[512:512 + g8n]
    fc2b_h = np.ascontiguousarray(fc2b_h.reshape(YCB, 128).T).astype(f32)

    # proj: rows in padded-576 layout (pad rows zero), cols plain C
    projwp = np.zeros((YCB * 128, C), f32)
    projwp[:CP] = rows_to_padded576(proj_w)
    projw_h = np.ascontiguousarray(
        projwp.reshape(YCB, 128, C).transpose(1, 0, 2)
    ).astype(BF16_NP)
    projb_h = np.ascontiguousarray(
        np.broadcast_to(proj_b[None, :], (128, C))
    ).astype(f32)

    # rw1: rows in the 6-column gate layout, scaled by 1/MEAN_N (folds the
    # mean).  Columns 0..4 match the padded-576 blocks (block 4 = g8 on the
    # lower partitions, fed by the h sums); column 5 holds g8 on the upper
    # partitions, fed by the w sums from the column-tiled pair.
    rw1s = rows_to_padded576(rw1_w / MEAN_N)          # [CP, C//4]
    rw1p = np.zeros((ZC * 128, C // 4), f32)
    rw1p[:512] = rw1s[:512]
    rw1p[512:512 + g8n] = rw1s[512:512 + g8n]
    rw1p[5 * 128 + 64:5 * 128 + 64 + g8n] = rw1s[512:512 + g8n]
    rw1w_h = np.ascontiguousarray(
        rw1p.reshape(ZC, 128, C // 4).transpose(1, 0, 2)
    ).astype(BF16_NP)
    rw1b_h = np.ascontiguousarray(rw1_b[:, None]).astype(f32)

    # rw2 columns: stream-0 logits in padded cols [0, CP), stream-1 logits in
    # padded cols [YCB*128, YCB*128 + CP) — so the device's 128-wide M-blocks
    # 0..4 are stream 0 and 5..9 are stream 1.
    NQ = YCB * 128
    rw2p = np.zeros((128, 2 * NQ), f32)
    rw2p[:, 0:CP][:, qvalid] = rw2_w[:, 2 * qidx[qvalid]]
    rw2p[:, NQ:NQ + CP][:, qvalid] = rw2_w[:, 2 * qidx[qvalid] + 1]
    rw2w_h = np.ascontiguousarray(rw2p).astype(BF16_NP)
    rw2b_full = np.zeros((2 * NQ,), f32)
    rw2b_full[0:CP][qvalid] = rw2_b[2 * qidx[qvalid]]
    rw2b_full[NQ:NQ + CP][qvalid] = rw2_b[2 * qidx[qvalid] + 1]
    rw2b_h = np.ascontiguousarray(rw2b_full.reshape(2 * YCB, 128).T).astype(f32)

    return dict(
        fcw=fcw_h, fcb=fcb_h, fc1w=fc1w_h, fc1b=fc1b_h, fc2w=fc2w_h,
        fc2b=fc2b_h, projw=projw_h, projb=projb_h, rw1w=rw1w_h, rw1b=rw1b_h,
        rw2w=rw2w_h, rw2b=rw2b_h,
    )


def _get_nc():
    if "nc" not in _CACHE:
        _CACHE["nc"] = build_nc()
    return _CACHE["nc"]


def run(inputs, trace=False, trace_kwargs=None):
    """Run the SPMD kernel; returns (full_output, BassKernelResults)."""
    x = np.asarray(inputs["x"], np.float32)
    shared = _prep_weights(
        np.asarray(inputs["fc_w"], np.float32), np.asarray(inputs["fc_b"], np.float32),
        np.asarray(inputs["fc1_w"], np.float32), np.asarray(inputs["fc1_b"], np.float32),
        np.asarray(inputs["fc2_w"], np.float32), np.asarray(inputs["fc2_b"], np.float32),
        np.asarray(inputs["rw1_w"], np.float32), np.asarray(inputs["rw1_b"], np.float32),
        np.asarray(inputs["rw2_w"], np.float32), np.asarray(inputs["rw2_b"], np.float32),
        np.asarray(inputs["proj_w"], np.float32), np.asarray(inputs["proj_b"], np.float32),
    )

    xf = x.reshape(B * T, HWTOK, C)
    in_maps = []
    for c in range(NCORES):
        sh = xf[NF * c:NF * (c + 1)]                      # [NF, 3136, 512]
        xt = sh.transpose(0, 2, 1).reshape(NF, CCB, 128, HWTOK)
        xt = np.ascontiguousarray(xt.transpose(0, 2, 1, 3)).astype(BF16_NP)
        m = dict(shared)
        m["xT"] = xt
        in_maps.append(m)

    nc = _get_nc()
    res = run_bass_kernel_spmd(
        nc, in_maps, list(range(NCORES)),
        trace=trace, **(dict(trace_kwargs=trace_kwargs) if trace_kwargs else {}),
    )

    out = np.empty((B * T, HWTOK, C), np.float32)
    for c in range(NCORES):
        out[NF * c:NF * (c + 1)] = res.results[c]["out"]
    return out.reshape(B, T, H, W, C), res


def kernel(**inputs) -> np.ndarray:
    full, _ = run(inputs, trace=False)
    return full
